# revision 28
# baseline (speedup 1.0000x reference)
"""Causal attention (LN -> QKV -> 16-head causal attn -> out-proj) on 8 TRN2 cores.

Sharding: core c = (batch b=c//4, head-group g=c%4). Each core runs its batch's
LayerNorm + a 4-head slice of QKV / attention / out-projection. The out-proj
partials (column-split over the inner dim) are summed on the host per batch.

v3 design notes (per core):
  - Weights pre-cast to bf16 and pre-permuted on the HOST (d = kb*P + p row
    order) so device DMAs are contiguous 128x4KB loads and no on-device casts
    are needed. First x blocks are DMA'd before the weights so LN starts ~2us.
  - ONE psum pool for the whole kernel (no pool-transition barriers): tag
    "ps" (QKV/V/transpose groups, 2 bufs x 1 bank), tag "s" (scores + out-proj,
    2 bufs x 2 banks), tag "po" (PV accumulators, 1 buf x 2 banks) = 16KB.
  - Attention runs in 512-wide q spans (quarters), DESCENDING (qq=3 first):
    the dense 16-key-block quarter directly follows phase A's dense QK
    stream, keeping the PE's ready queue deep (the clock governor only
    upclocks under sustained backlog). s_ps is double-buffered so S^T(kb+1)
    never waits on exp(kb); PV lags one key block behind S^T; exp covers
    both heads of the pair in one instruction.
  - V tiles carry a 65th all-ones column so PV psum row 64 accumulates the
    softmax denominators. Output normalization: quick evac (frees psum),
    reciprocal in a [128, 8] layout via DRAM shuffles, DRAM-broadcast,
    in-place multiply.
  - The out-projection of quarter qq+1 is emitted after quarter qq's
    attention (one-quarter lag), so normalization chains are off the PE
    critical path and output DMAs spread across phase B.
"""

import numpy as np
import ml_dtypes

import concourse.bass as bass
import concourse.mybir as mybir
import concourse.tile as tile
from concourse import bacc
from concourse.bass_utils import run_bass_kernel_spmd
from concourse.masks import make_identity

B, N, DIM, HEADS, DIM_HEAD = 2, 2048, 1024, 16, 64
INNER = HEADS * DIM_HEAD
H_LOC = 4                      # heads per core
N_CORES = 8
P = 128
NB = N // P                    # 16 seq blocks
KB = DIM // P                  # 8 dim blocks
QS = 512                       # q span per attention block (one psum bank)
NQ = N // QS                   # 4 quarters
SCALE = DIM_HEAD ** -0.5
LN_EPS = 1e-5

F32 = mybir.dt.float32
BF16 = mybir.dt.bfloat16
AF = mybir.ActivationFunctionType
ALU = mybir.AluOpType


def build_nc():
    from contextlib import ExitStack

    nc = bacc.Bacc(None, target_bir_lowering=False, debug=False)

    x_d = nc.dram_tensor("x", [N, DIM], F32, kind="ExternalInput")
    wq_d = nc.dram_tensor("wq", [P, KB, H_LOC * DIM_HEAD], BF16, kind="ExternalInput")
    wk_d = nc.dram_tensor("wk", [P, KB, H_LOC * DIM_HEAD], BF16, kind="ExternalInput")
    wv_d = nc.dram_tensor("wv", [P, KB, H_LOC * DIM_HEAD], BF16, kind="ExternalInput")
    wo_d = nc.dram_tensor("wo", [P, 2, DIM], BF16, kind="ExternalInput")
    bq_d = nc.dram_tensor("bq", [P, 2], F32, kind="ExternalInput")
    bk_d = nc.dram_tensor("bk", [P, 2], F32, kind="ExternalInput")
    bv_d = nc.dram_tensor("bv", [1, H_LOC * DIM_HEAD], F32, kind="ExternalInput")
    out_d = nc.dram_tensor("out", [N, DIM], F32, kind="ExternalOutput")

    with tile.TileContext(nc) as tc:
        ctx = ExitStack()
        with ctx:
            const = ctx.enter_context(tc.tile_pool(name="const", bufs=1))
            persist = ctx.enter_context(tc.tile_pool(name="persist", bufs=1))
            xpool = ctx.enter_context(tc.tile_pool(name="xpool", bufs=5))
            xnpool = ctx.enter_context(tc.tile_pool(name="xnpool", bufs=4))
            stat = ctx.enter_context(tc.tile_pool(name="stat", bufs=8))
            expp = ctx.enter_context(tc.tile_pool(name="expp", bufs=4))
            drp = ctx.enter_context(tc.tile_pool(name="drp", bufs=4))
            rbcp = ctx.enter_context(tc.tile_pool(name="rbcp", bufs=2))
            dramp = ctx.enter_context(tc.tile_pool(name="dramp", bufs=6, space="DRAM"))
            stage = ctx.enter_context(tc.tile_pool(name="stage", bufs=3))
            psum = ctx.enter_context(tc.tile_pool(name="psum", bufs=1, space="PSUM"))

            # ---- first x blocks before the weight loads; each block loads
            # as two half-DMAs (two queues -> half the arrival latency, and
            # bn_stats(a=0) only needs the first half)
            x_ts = {}

            def load_x(sb, chunks=2):
                t = xpool.tile([P, DIM], F32, tag="x", name=f"x{sb}")
                w = DIM // chunks
                for h in range(chunks):
                    nc.sync.dma_start(
                        t[:, h * w:(h + 1) * w],
                        x_d[sb * P:(sb + 1) * P, h * w:(h + 1) * w],
                    )
                return t

            for sb in range(4):
                x_ts[sb] = load_x(sb, chunks=4 if sb < 2 else 2)

            # ---- constants / weights (bf16, host-permuted, contiguous) ----
            eps_t = const.tile([P, 1], F32, tag="eps")
            nc.vector.memset(eps_t, LN_EPS)
            bq_sb = const.tile([P, 2], F32, tag="bq")
            nc.sync.dma_start(bq_sb[:], bq_d[:])
            bk_sb = const.tile([P, 2], F32, tag="bk")
            nc.sync.dma_start(bk_sb[:], bk_d[:])
            bv_sb = const.tile([P, H_LOC, DIM_HEAD], F32, tag="bv")
            nc.sync.dma_start(
                bv_sb[:],
                bv_d[:].rearrange("o (h d) -> o h d", h=H_LOC)
                .to_broadcast((P, H_LOC, DIM_HEAD)),
            )
            wv_bf = persist.tile([P, KB, H_LOC * DIM_HEAD], BF16, tag="wv")
            nc.sync.dma_start(wv_bf[:], wv_d[:])
            wq_bf = persist.tile([P, KB, H_LOC * DIM_HEAD], BF16, tag="wq")
            nc.sync.dma_start(wq_bf[:], wq_d[:])
            wk_bf = persist.tile([P, KB, H_LOC * DIM_HEAD], BF16, tag="wk")
            nc.sync.dma_start(wk_bf[:], wk_d[:])
            wo_bf = persist.tile([P, 2, DIM], BF16, tag="wo")
            nc.sync.dma_start(wo_bf[:], wo_d[:])

            ident = const.tile([P, P], BF16, tag="ident")
            make_identity(nc, ident)
            # keep-mask for the causal diagonal block, both heads of a pair:
            # tri3[k, a, q] = (k <= q)
            tri3 = const.tile([P, 2, P], BF16, tag="tri3")
            nc.gpsimd.memset(tri3[:], 0.0)
            nc.gpsimd.affine_select(
                out=tri3[:], in_=tri3[:], compare_op=ALU.is_gt, fill=1.0,
                base=0, channel_multiplier=1, pattern=[[0, 2], [-1, P]],
            )

            # xnT quarters: xnT[q][p, j, kb, s] = xn[(4q+j)*P + s, kb*P + p]
            xnT = [persist.tile([P, 4, KB, P], BF16, tag=f"xnT{q}", name=f"xnT{q}")
                   for q in range(4)]
            QTt = [persist.tile([P, N], BF16, tag=f"qt{p_}", name=f"qt{p_}")
                   for p_ in range(2)]
            KTt = [persist.tile([P, N], BF16, tag=f"kt{p_}", name=f"kt{p_}")
                   for p_ in range(2)]
            Vt = persist.tile([P, NB, H_LOC, DIM_HEAD + 1], BF16, tag="v")
            nc.gpsimd.memset(Vt[:], 1.0)  # 65th column stays 1.0 -> denominators
            outT = [[persist.tile([P, QS], BF16, tag=f"outT{p_}_{q_}",
                                  name=f"outT{p_}_{q_}") for q_ in range(NQ)]
                    for p_ in range(2)]

            # ---- phase A: LN -> transpose -> QKV -> V (interleaved) ----
            def emit_qkv_st(st):
                for (wt, bias_sb, dstt) in ((wq_bf, bq_sb, QTt), (wk_bf, bk_sb, KTt)):
                    for pr in range(2):
                        ps = psum.tile([P, 512], F32, tag="ps", bufs=2)
                        for kb in range(KB):
                            nc.tensor.matmul(
                                ps[:],
                                wt[:, kb, pr * P:(pr + 1) * P],
                                xnT[st][:, :, kb, :],
                                start=(kb == 0), stop=(kb == KB - 1),
                            )
                        # bias-add evacuation on ScalarE (per-partition bias)
                        nc.scalar.activation(
                            dstt[pr][:, st * 512:(st + 1) * 512], ps[:],
                            AF.Identity, bias=bias_sb[:, pr:pr + 1],
                        )

            for sb in range(NB):
                if sb + 4 < NB:
                    x_ts[sb + 4] = load_x(sb + 4)
                x_t = x_ts.pop(sb)

                stats = stat.tile([P, 2, 6], F32, tag="bnst")
                x3 = x_t[:].rearrange("p (a f) -> p a f", a=2)
                for a in range(2):
                    nc.vector.bn_stats(stats[:, a, :], x3[:, a, :])
                mv = stat.tile([P, 2], F32, tag="mv")
                nc.vector.bn_aggr(mv[:], stats[:])
                rstd = stat.tile([P, 1], F32, tag="rstd")
                nc.scalar.activation(rstd[:], mv[:, 1:2], AF.Sqrt, bias=eps_t[:])
                if sb == NB - 1:
                    # pre-load the Exp table while the PE chews through the
                    # phase-A tail. Input-dependent on the LAST Sqrt so the
                    # scheduler cannot hoist it before the Sqrt stream.
                    warm = stat.tile([P, 1], F32, tag="warm")
                    nc.scalar.activation(warm[:], rstd[:], AF.Exp)
                nc.vector.reciprocal(rstd[:], rstd[:])
                # nmrs = -mean * rstd  -> xn = x*rstd + nmrs on ScalarE
                nmrs = stat.tile([P, 1], F32, tag="nmrs")
                nc.vector.tensor_scalar(
                    nmrs[:], mv[:, 0:1], rstd[:], -1.0, ALU.mult, ALU.mult
                )
                xn_bf = xnpool.tile([P, DIM], BF16, tag="xn")
                nc.scalar.activation(
                    xn_bf[:], x_t[:], AF.Identity, bias=nmrs[:], scale=rstd[:]
                )

                # transpose this seq block: 8 dim-blocks via PE, 2 psum tiles
                for half in range(2):
                    ps = psum.tile([P, 512], F32, tag="ps", bufs=2)
                    for j in range(4):
                        kb = half * 4 + j
                        nc.tensor.matmul(
                            ps[:, j * P:(j + 1) * P],
                            xn_bf[:, kb * P:(kb + 1) * P],
                            ident[:],
                            start=True, stop=True,
                        )
                    dst = xnT[sb // 4][:, sb % 4, half * 4:(half + 1) * 4, :]
                    src = ps[:].rearrange("p (a f) -> p a f", a=4)
                    if half == 0:
                        nc.scalar.copy(dst, src)
                    else:
                        nc.vector.tensor_copy(dst, src)

                # V for this seq block
                ps = psum.tile([P, 512], F32, tag="ps", bufs=2)
                psv = ps[:, :H_LOC * DIM_HEAD]
                for kb in range(KB):
                    nc.tensor.matmul(
                        psv,
                        xnT[sb // 4][:, sb % 4, kb, :],
                        wv_bf[:, kb, :],
                        start=(kb == 0), stop=(kb == KB - 1),
                    )
                nc.vector.tensor_tensor(
                    Vt[:, sb, :, :DIM_HEAD],
                    psv.rearrange("p (h d) -> p h d", h=H_LOC),
                    bv_sb[:],
                    ALU.add,
                )

                if sb % 4 == 3:
                    emit_qkv_st(sb // 4)

            # ---- phase B: attention in q quarters (descending), with the
            # out-projection of the previous quarter folded in ----
            def emit_outproj(qq):
                # uses the phase-A "ps" psum tag (idle during phase B) so the
                # attention blocks' "s" rotation never WAR-stalls on these;
                # stores split in 2 half-DMAs to halve per-queue latency
                for qb in range(qq * 4, qq * 4 + 4):
                    for nt in range(2):
                        ps = psum.tile([P, 512], F32, tag="ps", bufs=2,
                                       name=f"pp{qb}_{nt}")
                        for pb in range(2):
                            nc.tensor.matmul(
                                ps[:],
                                outT[pb][qb // 4][:, (qb % 4) * P:(qb % 4 + 1) * P],
                                wo_bf[:, pb, nt * 512:(nt + 1) * 512],
                                start=(pb == 0), stop=(pb == 1),
                            )
                        so = stage.tile([P, 512], F32, tag="so",
                                        name=f"so{qb}_{nt}", bufs=8)
                        if nt == 0:
                            nc.scalar.copy(so[:], ps[:])
                        else:
                            nc.vector.tensor_copy(so[:], ps[:])
                        for qd in range(2):
                            nc.sync.dma_start(
                                out_d[qb * P:(qb + 1) * P,
                                      nt * 512 + qd * 256:nt * 512 + (qd + 1) * 256],
                                so[:, qd * 256:(qd + 1) * 256],
                            )

            for qi, qq in enumerate(reversed(range(NQ))):
                qs, qe = qq * QS, (qq + 1) * QS
                nkb = qe // P
                for pr in range(2):
                    ps_o = psum.tile([DIM_HEAD + 1, 2, QS], F32, tag="po",
                                     bufs=1, name=f"po_{pr}_{qq}")

                    def emit_pv(kb, ex):
                        qlo = kb * P
                        cs = max(qlo, qs)
                        for hh in range(2):
                            nc.tensor.matmul(
                                ps_o[:, hh, cs - qs:],
                                Vt[:, kb, 2 * pr + hh, :],
                                ex[:, hh, cs - qs:],
                                start=(kb == 0),
                                stop=(kb == nkb - 1),
                            )

                    prev = None  # (kb, ex) with PV not yet emitted
                    for kb in range(nkb):
                        qlo = kb * P
                        vstart = max(qlo, qs)
                        s_ps = psum.tile([P, 2, QS], F32, tag="s", bufs=2,
                                         name=f"s_{pr}_{qq}_{kb}")
                        for hh in range(2):
                            po = hh * DIM_HEAD
                            nc.tensor.matmul(
                                s_ps[:, hh, vstart - qs:],
                                KTt[pr][po:po + DIM_HEAD, qlo:qlo + P],
                                QTt[pr][po:po + DIM_HEAD, vstart:qe],
                                start=True, stop=True,
                                tile_position=(po, 0),
                            )
                        ex = expp.tile([P, 2, QS], BF16, tag="ex",
                                       name=f"ex_{pr}_{qq}_{kb}")
                        nc.scalar.activation(
                            ex[:, :, vstart - qs:],
                            s_ps[:, :, vstart - qs:],
                            AF.Exp,
                        )
                        if qlo >= qs:
                            nc.vector.tensor_tensor(
                                ex[:, :, qlo - qs:qlo - qs + P],
                                ex[:, :, qlo - qs:qlo - qs + P],
                                tri3[:],
                                ALU.mult,
                            )
                        if prev is not None:
                            emit_pv(*prev)
                        prev = (kb, ex)
                    emit_pv(*prev)

                    # evacuate [65, QS] per head on DVE — the denominator row
                    # rides along, so ScalarE does nothing but exp in phase B.
                    # Normalize: bf16 reciprocal in [128, 8] via DRAM
                    # shuffles (DVE recip cost scales with free size), then
                    # per-head multiply outT = o65 * recip_bc (2x DVE mode).
                    da = dramp.tile([2, QS], BF16, tag="da",
                                    name=f"da{pr}_{qq}")
                    o65s = []
                    for hh in range(2):
                        o65 = drp.tile([DIM_HEAD + 1, QS], BF16, tag=f"o65_{hh}",
                                       name=f"o65_{pr}_{qq}_{hh}")
                        nc.vector.tensor_copy(o65[:], ps_o[:, hh, :])
                        nc.sync.dma_start(da[hh:hh + 1, :],
                                          o65[DIM_HEAD:DIM_HEAD + 1, :])
                        o65s.append(o65)
                    dsh = drp.tile([P, 2, QS // P], BF16, tag="dsh",
                                   name=f"dsh{pr}_{qq}")
                    nc.sync.dma_start(
                        dsh[:],
                        da[:].rearrange("h (p o) -> p h o", o=QS // P),
                    )
                    with nc.allow_low_precision(
                            reason="softmax denominators tolerate bf16"):
                        nc.vector.reciprocal(dsh[:], dsh[:])
                    db = dramp.tile([2, QS], BF16, tag="db",
                                    name=f"db{pr}_{qq}")
                    nc.sync.dma_start(
                        db[:].rearrange("h (p o) -> p h o", o=QS // P),
                        dsh[:],
                    )
                    for hh in range(2):
                        rbc = rbcp.tile([DIM_HEAD, QS], BF16, tag=f"rbc{hh}",
                                        name=f"rbc{pr}_{qq}_{hh}")
                        nc.sync.dma_start(
                            rbc[:],
                            db[hh:hh + 1, :].to_broadcast((DIM_HEAD, QS)),
                        )
                        nc.vector.tensor_tensor(
                            outT[pr][qq][hh * DIM_HEAD:(hh + 1) * DIM_HEAD, :],
                            o65s[hh][:DIM_HEAD, :],
                            rbc[:],
                            ALU.mult,
                        )

                if qi > 0:
                    emit_outproj(list(reversed(range(NQ)))[qi - 1])
            emit_outproj(0)

    nc.compile()
    return nc


def make_in_maps(x, ln_w, ln_b, w_qkv, w_out):
    x = np.asarray(x, np.float32)
    ln_w = np.asarray(ln_w, np.float32)
    ln_b = np.asarray(ln_b, np.float32)
    w_qkv = np.asarray(w_qkv, np.float32)
    w_out = np.asarray(w_out, np.float32)
    bf16 = ml_dtypes.bfloat16

    def perm_w(w):
        # device row (p, kb) holds dim d = kb*P + p (PE transpose layout)
        return np.ascontiguousarray(
            w.reshape(KB, P, -1).transpose(1, 0, 2)).astype(bf16)

    in_maps = []
    for c in range(N_CORES):
        b, g = c // 4, c % 4
        cols = np.arange(4 * g * DIM_HEAD, (4 * g + H_LOC) * DIM_HEAD)
        wq_s = w_qkv[:, cols]
        wk_s = w_qkv[:, INNER + cols]
        wv_s = w_qkv[:, 2 * INNER + cols]
        wq = perm_w(ln_w[:, None] * wq_s * SCALE)
        wk = perm_w(ln_w[:, None] * wk_s)
        wv = perm_w(ln_w[:, None] * wv_s)
        wo = np.ascontiguousarray(
            w_out[cols, :].reshape(2, P, DIM).transpose(1, 0, 2)).astype(bf16)
        bq = (ln_b @ wq_s) * SCALE
        bk = ln_b @ wk_s
        bv = ln_b @ wv_s
        in_maps.append({
            "x": np.ascontiguousarray(x[b]),
            "wq": wq, "wk": wk, "wv": wv, "wo": wo,
            "bq": np.ascontiguousarray(bq.reshape(2, P).T),
            "bk": np.ascontiguousarray(bk.reshape(2, P).T),
            "bv": bv.reshape(1, H_LOC * DIM_HEAD),
        })
    return in_maps


_NC_CACHE = []


def kernel(x, ln_w, ln_b, w_qkv, w_out):
    in_maps = make_in_maps(x, ln_w, ln_b, w_qkv, w_out)
    if not _NC_CACHE:
        _NC_CACHE.append(build_nc())
    nc = _NC_CACHE[0]
    res = run_bass_kernel_spmd(nc, in_maps, list(range(N_CORES))).results
    out = np.zeros((B, N, DIM), np.float32)
    for c in range(N_CORES):
        out[c // 4] += res[c]["out"]
    return out
